# revision 11
# baseline (speedup 1.0000x reference)
"""Trainium2 Bass kernel for nn_PreTrainModel_4355096838991 (gnn_message_passing).

2-layer hetero GraphConv (R=20 directed etypes) + link-prediction classifier.

Strategy (8 NeuronCores, SPMD single program):
  * Host (index-only prep): degree norms folded into per-edge weight
    w_e = src_norm[r,src]*dst_norm[r,dst]/R; all feature tensors pre-cast
    to bf16; dst nodes assigned to (core, 250-wide block) with a
    swap-repair balancer so every (core,block,rel) group fits kt=3 tiles
    of 128 edge slots (25% fewer gather descriptors than naive 125-wide
    blocks); pad slots fetch row 0 with w=0.
  * Device layer: dma_gather h[src] rows (256-col bf16 table), gathers
    alternating over 2 SWDGE queues to overlap per-descriptor HBM
    latency -> weighted one-hot O_w = (iota==dstloc)*w via one fused
    tensor_scalar per 128-slot tile -> scatter matmuls into PSUM
    accT[d_chunk, dst 250] -> transform accT @ W_r (two 125-halves)
    accumulated over relations in PSUM -> +mean-bias -> staged ->
    AllGather.
  * Layer 2 computes only the ~2048 queried nodes (sub/obj union).
  * Classifier: vocab-sharded GEMMs, features transposed via PE; bias and
    the rel-emb ones-column folded into the 4-chunk weight stack; bf16
    logit writes, host casts back to f32.
"""

import sys

for _p in ("/opt/trn_rl_repo/concourse", "/opt/trn_rl_repo"):
    if _p not in sys.path:
        sys.path.insert(0, _p)

import numpy as np
import ml_dtypes

import concourse.bacc as bacc
import concourse.bass as bass
import concourse.tile as tile
from concourse import mybir
from concourse.bass_utils import run_bass_kernel_spmd
from concourse.masks import make_identity

NC = 8
DP = 256  # padded feature row (bytes multiple of 256 for dma_gather)
TS_ = 128  # edge sub-tile (scatter matmul contraction K); 64 or 128
PROFILE = False
RUN_KWARGS = {}

_CACHE = {}
bf16 = ml_dtypes.bfloat16


def _largest_div(n, cap):
    for d in range(cap, 0, -1):
        if n % d == 0:
            return d
    return 1


class Cfg:
    pass


def _make_cfg(nume, numr, din, e, b):
    c = Cfg()
    c.nume, c.numr, c.r, c.d, c.e, c.b = nume, numr, 2 * numr, din, e, b
    assert din == 200 and nume % NC == 0 and nume <= 32000
    c.fdt = mybir.dt.bfloat16
    c.nshard = nume // NC
    c.blk1 = _largest_div(c.nshard, 250)     # 250 (double dst-block)
    c.nblk1 = c.nshard // c.blk1             # 10
    c.c0 = min(128, din)
    c.c1 = din - c.c0
    c.vshard = nume // NC
    c.nqt = b // 128
    assert b % 128 == 0
    c.vts = []
    off = 0
    while off < c.vshard:
        w = min(512, c.vshard - off)
        c.vts.append((off, w))
        off += w
    return c


def _balance(iv, nblocks, bsize, limit, iters=400):
    """Assign rows of iv [N, R] to nblocks blocks of exactly bsize each,
    swap-repairing until every (block, r) sum <= limit (best effort)."""
    n = iv.shape[0]
    assert n == nblocks * bsize
    assign = np.repeat(np.arange(nblocks), bsize)
    S = np.zeros((nblocks, iv.shape[1]), np.int64)
    np.add.at(S, assign, iv)
    for _ in range(iters):
        if S.max() <= limit:
            break
        b, r = np.unravel_index(S.argmax(), S.shape)
        mem = np.where(assign == b)[0]
        n_h = mem[iv[mem, r].argmax()]
        b2 = int(S[:, r].argmin())
        if b2 == b:
            break
        mem2 = np.where(assign == b2)[0]
        n_l = mem2[iv[mem2, r].argmin()]
        assign[n_h], assign[n_l] = b2, b
        S[b] += iv[n_l] - iv[n_h]
        S[b2] += iv[n_h] - iv[n_l]
    return assign


def _streams(gkey, src, w, dloc, kt, ngroups_per_core):
    """gkey: global group id per edge; groups padded to kt sub-tiles of 64."""
    slots_g = kt * TS_
    t64 = ngroups_per_core * kt
    slots = t64 * TS_
    assert (NC * slots) % 128 == 0
    order = np.argsort(gkey, kind="stable")
    ks = gkey[order]
    ngroups = NC * ngroups_per_core
    grp_start = np.searchsorted(ks, np.arange(ngroups))
    cnt = np.diff(np.append(grp_start, len(ks)))
    assert cnt.max() <= slots_g, (cnt.max(), slots_g)
    rank = np.arange(len(ks)) - grp_start[ks]
    glocal = ks % ngroups_per_core
    core_of = ks // ngroups_per_core
    pos = core_of * slots + glocal * slots_g + rank
    SRC = np.zeros(NC * slots, np.int16)
    WV = np.zeros(NC * slots, np.float32)
    DL = np.zeros(NC * slots, np.float32)
    SRC[pos] = src[order].astype(np.int16)
    WV[pos] = w[order]
    DL[pos] = dloc[order].astype(np.float32)
    t = slots // 128  # 128-columns per core
    SRC = SRC.reshape(NC, t, 128)
    WV = WV.reshape(NC, t, 128)
    DL = DL.reshape(NC, t, 128)
    idx = np.zeros((NC, 128, t * 8), np.int16)
    for c in range(NC):
        blk = SRC[c].reshape(t * 8, 16).T
        for k in range(8):  # replicate for the 8 GPSIMD descriptor cores
            idx[c, 16 * k:16 * k + 16] = blk
    wv = np.transpose(WV, (0, 2, 1)).copy()  # [NC, 128, T]
    dl = np.transpose(DL, (0, 2, 1)).copy()
    return idx, wv, dl, t


def _host_prep(cfg, sub, obj, rel, src, dst, etype):
    nume, R = cfg.nume, cfg.r
    seg_s = etype.astype(np.int64) * nume + src
    seg_d = etype.astype(np.int64) * nume + dst
    outdeg = np.bincount(seg_s, minlength=R * nume).astype(np.float32)
    indeg = np.bincount(seg_d, minlength=R * nume).astype(np.float32)
    srcn = np.maximum(outdeg, 1.0) ** -0.5
    dstn = np.maximum(indeg, 1.0) ** -0.5
    w = (srcn[seg_s] * dstn[seg_d] / R).astype(np.float32)

    iv = np.bincount(seg_d, minlength=R * nume).reshape(R, nume).T  # [N, R]

    # --- layer 1: balanced dst-node -> (core, block) assignment ---
    nblocks = NC * cfg.nblk1
    assignb = _balance(iv.astype(np.int64), nblocks, cfg.blk1, 3 * 128 - 6)
    # position of each node within the permuted table
    orderp = np.argsort(assignb, kind="stable")
    invpos = np.empty(nume, np.int64)
    invpos[orderp] = np.arange(nume)

    core1 = assignb[dst] // cfg.nblk1
    b1 = assignb[dst] % cfg.nblk1
    dloc1 = invpos[dst] % cfg.blk1
    gpc1 = cfg.nblk1 * R
    gkey1 = (core1.astype(np.int64) * cfg.nblk1 + b1) * R + etype
    cnt1 = np.bincount(gkey1, minlength=NC * gpc1)
    kt1 = int(-(-cnt1.max() // TS_))
    idx1, wv1, dl1, t1 = _streams(gkey1, src, w, dloc1, kt1, gpc1)

    # --- layer 2: only queried dst nodes, balanced into 16 q-blocks ---
    qn = np.unique(np.concatenate([sub, obj]))
    nq = len(qn)
    nblk2 = max(1, -(-nq // (128 * NC)))
    qshard = nblk2 * 128
    qpad = NC * qshard
    ivq = np.zeros((qpad, R), np.int64)
    ivq[:nq] = iv[qn]
    assignq = _balance(ivq, NC * nblk2, 128, 188)
    orderq = np.argsort(assignq, kind="stable")
    qpos_of = np.empty(qpad, np.int64)
    qpos_of[orderq] = np.arange(qpad)
    qmap = np.full(nume, -1, np.int64)
    qmap[qn] = qpos_of[np.arange(nq)]
    m = qmap[dst] >= 0
    q = qmap[dst[m]]
    core2 = q // qshard
    loc2 = q % qshard
    b2 = loc2 // 128
    dloc2 = loc2 % 128
    gpc2 = nblk2 * R
    gkey2 = (core2 * nblk2 + b2) * R + etype[m]
    cnt2 = np.bincount(gkey2, minlength=NC * gpc2)
    kt2 = max(2, int(-(-cnt2.max() // TS_)))
    # layer-2 gathers read the permuted h1 table
    idx2, wv2, dl2, t2 = _streams(gkey2, invpos[src[m]], w[m], dloc2, kt2, gpc2)

    # --- classifier query index streams (int16, [128, B/16] rows 0..15) ---
    def qidx(a):
        out = np.zeros((128, cfg.b // 16), np.int16)
        blk = a.astype(np.int16).reshape(cfg.b // 16, 16).T
        for k in range(8):
            out[16 * k:16 * k + 16] = blk
        return out

    meta = dict(
        kt1=kt1, t1=t1, kt2=kt2, t2=t2, nblk2=nblk2, qshard=qshard, qpad=qpad,
        idx1=idx1, wv1=wv1, dl1=dl1, idx2=idx2, wv2=wv2, dl2=dl2,
        qobj=qidx(qmap[obj]), qsub=qidx(qmap[sub]), qrel=qidx(rel),
        orderp=orderp,
    )
    return meta


# ----------------------------------------------------------------------------
# device program
# ----------------------------------------------------------------------------

def _build_nc(cfg):
    fdt = cfg.fdt
    f32 = mybir.dt.float32
    i16 = mybir.dt.int16
    R, D, BLK1 = cfg.r, cfg.d, cfg.blk1
    c0, c1 = cfg.c0, cfg.c1
    gb1 = _largest_div(cfg.t1, 8)  # dma_gather HW limit: 1024 idxs
    gb2 = _largest_div(cfg.t2, 8)

    nc = bacc.Bacc("TRN2", target_bir_lowering=False, debug=False, num_devices=NC,
                   num_swdge_queues=2)

    # ---- I/O (all feature data pre-cast to bf16 on host) ----
    entp = nc.dram_tensor("entp", [cfg.nume, DP], fdt, kind="ExternalInput")
    wch = nc.dram_tensor("wch", [2, D, R * D], fdt, kind="ExternalInput")
    b0 = nc.dram_tensor("b0", [R, D], f32, kind="ExternalInput")
    b1 = nc.dram_tensor("b1", [R, D], f32, kind="ExternalInput")
    wsca = nc.dram_tensor("wsca", [512, cfg.vshard], fdt, kind="ExternalInput")
    woca = nc.dram_tensor("woca", [512, cfg.vshard], fdt, kind="ExternalInput")
    srelp = nc.dram_tensor("srelp", [cfg.numr, DP], fdt, kind="ExternalInput")
    orelp = nc.dram_tensor("orelp", [cfg.numr, DP], fdt, kind="ExternalInput")
    idx1 = nc.dram_tensor("idx1", [128, cfg.t1 * 8], i16, kind="ExternalInput")
    wv1 = nc.dram_tensor("wv1", [128, cfg.t1], f32, kind="ExternalInput")
    dl1 = nc.dram_tensor("dl1", [128, cfg.t1], f32, kind="ExternalInput")
    idx2 = nc.dram_tensor("idx2", [128, cfg.t2 * 8], i16, kind="ExternalInput")
    wv2 = nc.dram_tensor("wv2", [128, cfg.t2], f32, kind="ExternalInput")
    dl2 = nc.dram_tensor("dl2", [128, cfg.t2], f32, kind="ExternalInput")
    qobj = nc.dram_tensor("qobj", [128, cfg.b // 16], i16, kind="ExternalInput")
    qsub = nc.dram_tensor("qsub", [128, cfg.b // 16], i16, kind="ExternalInput")
    qrel = nc.dram_tensor("qrel", [128, cfg.b // 16], i16, kind="ExternalInput")
    slog = nc.dram_tensor("slog", [cfg.b, cfg.vshard], fdt, kind="ExternalOutput")
    olog = nc.dram_tensor("olog", [cfg.b, cfg.vshard], fdt, kind="ExternalOutput")

    h1loc = nc.dram_tensor("h1loc", [cfg.nshard, DP], fdt)
    h1all = nc.dram_tensor("h1all", [cfg.nume, DP], fdt, addr_space="Shared")
    h2loc = nc.dram_tensor("h2loc", [cfg.qshard, DP], fdt)
    h2all = nc.dram_tensor("h2all", [cfg.qpad, DP], fdt, addr_space="Shared")

    groups = [list(range(NC))]

    with tile.TileContext(nc) as tc:
        from contextlib import ExitStack

        with ExitStack() as ctx:
            const = ctx.enter_context(tc.tile_pool(name="const", bufs=1))

            # constants
            iota_i = const.tile([128, 256], mybir.dt.int32)
            nc.gpsimd.iota(iota_i[:], pattern=[[1, 256]], base=0, channel_multiplier=0)
            iota_f = const.tile([128, 256], fdt)
            nc.vector.tensor_copy(iota_f[:], iota_i[:])

            # W chunk tiles (pre-cast on host, plain HWDGE loads)
            wc0 = [const.tile([c0, R * D], fdt, tag=f"wc0_{l}", name=f"wc0_{l}") for l in range(2)]
            wc1 = [const.tile([c1, R * D], fdt, tag=f"wc1_{l}", name=f"wc1_{l}") for l in range(2)]
            for l in range(2):
                nc.sync.dma_start(out=wc0[l][:], in_=wch[l, 0:c0, :])
                nc.sync.dma_start(out=wc1[l][:], in_=wch[l, c0:D, :])

            # metadata
            def meta_tiles(idx_d, wv_d, dl_d, t, nm):
                it = const.tile([128, t * 8], i16, tag=f"it{nm}", name=f"it{nm}")
                nc.sync.dma_start(out=it[:], in_=idx_d[:])
                wt = const.tile([128, t], f32, tag=f"wt{nm}", name=f"wt{nm}")
                nc.sync.dma_start(out=wt[:], in_=wv_d[:])
                dt_ = const.tile([128, t], f32, tag=f"dt{nm}", name=f"dt{nm}")
                nc.sync.dma_start(out=dt_[:], in_=dl_d[:])
                return it, wt, dt_

            it1, wt1, dt1 = meta_tiles(idx1, wv1, dl1, cfg.t1, "1")
            it2, wt2, dt2 = meta_tiles(idx2, wv2, dl2, cfg.t2, "2")

            stage1 = const.tile([128, cfg.nblk1 * max(1, cfg.blk1 // 128 + (1 if cfg.blk1 % 128 else 0)), DP], fdt)
            nc.vector.memset(stage1[:], 0.0)
            stage2 = const.tile([128, cfg.nblk2, DP], fdt)
            nc.vector.memset(stage2[:], 0.0)

            with (
                tc.tile_pool(name="lp", bufs=2) as lp,
                tc.tile_pool(name="l2p", bufs=2) as l2p,
                tc.tile_pool(name="ow", bufs=4) as owp,
                tc.tile_pool(name="acc", bufs=3) as accp,
                tc.tile_pool(name="ps", bufs=2, space="PSUM") as psp,
            ):
                # mean bias, replicated over partitions: ones[R,128].T @ b
                bbar = []
                ones_r = const.tile([R, 128], f32)
                nc.vector.memset(ones_r[:], 1.0 / R)
                for l, bt in enumerate((b0, b1)):
                    bsb = const.tile([R, D], f32, tag=f"bsb{l}", name=f"bsb{l}")
                    nc.sync.dma_start(out=bsb[:], in_=bt[:])
                    bps = psp.tile([128, D], f32, tag="outp", name="outp")
                    nc.tensor.matmul(bps[:], lhsT=ones_r[:], rhs=bsb[:], start=True, stop=True)
                    brep = const.tile([128, D], f32, tag=f"brep{l}", name=f"brep{l}")
                    nc.scalar.copy(brep[:], bps[:])
                    bbar.append(brep)

                def layer(tbl, it, wt, dt_, kt, t, gb, nblk, blk, wc0l, wc1l, brep,
                          stage, out_dram, pool):
                    nbat = t // gb
                    acc0 = acc1 = outp = None
                    for g in range(nbat):
                        msg = pool.tile([128, gb, DP], fdt, tag="msg", name="msg")
                        nc.gpsimd.dma_gather(
                            out_ap=msg[:],
                            in_ap=tbl[:],
                            idxs_ap=it[:, g * gb * 8:(g + 1) * gb * 8],
                            num_idxs=gb * 128,
                            num_idxs_reg=gb * 128,
                            elem_size=DP,
                            queue_num=g % 2,
                        )
                        for j in range(gb):
                            col = g * gb + j
                            ow = owp.tile([128, blk], fdt, tag="ow", name="ow")
                            nc.vector.tensor_scalar(
                                out=ow[:], in0=iota_f[:, :blk],
                                scalar1=dt_[:, col:col + 1], scalar2=wt[:, col:col + 1],
                                op0=mybir.AluOpType.is_equal, op1=mybir.AluOpType.mult,
                            )
                            for half in range(128 // TS_):
                                st = col * (128 // TS_) + half
                                grp = st // kt
                                k = st % kt
                                bb = grp // R
                                r = grp % R
                                if k == 0:
                                    acc0 = psp.tile([128, blk], f32, tag="accT0", name="accT0")
                                    acc1 = psp.tile([128, blk], f32, tag="accT1", name="accT1")
                                h0 = half * TS_
                                nc.tensor.matmul(acc0[:], lhsT=msg[h0:h0 + TS_, j, 0:c0],
                                                 rhs=ow[h0:h0 + TS_, :],
                                                 start=(k == 0), stop=(k == kt - 1))
                                # cols c0:c0+128 include the zero-padded tail so the
                                # full 128 psum partitions are written (rows >= c1: 0)
                                nc.tensor.matmul(acc1[:], lhsT=msg[h0:h0 + TS_, j, c0:c0 + 128],
                                                 rhs=ow[h0:h0 + TS_, :],
                                                 start=(k == 0), stop=(k == kt - 1))
                                if k == kt - 1:
                                    asb0 = accp.tile([128, blk], fdt, tag="asb0", name="asb0")
                                    asb1 = accp.tile([128, blk], fdt, tag="asb1", name="asb1")
                                    if grp % 2 == 0:
                                        nc.scalar.copy(asb0[:], acc0[:])
                                        nc.scalar.copy(asb1[:], acc1[:])
                                    else:
                                        nc.vector.tensor_copy(asb0[:], acc0[:])
                                        nc.vector.tensor_copy(asb1[:], acc1[:])
                                    nh = 1 if blk <= 128 else 2
                                    hb = blk // nh
                                    if r == 0:
                                        outp = psp.tile([hb, nh, D], f32, tag="outp", name="outp")
                                    for h in range(nh):
                                        csl = slice(h * hb, (h + 1) * hb)
                                        nc.tensor.matmul(outp[:, h, :], lhsT=asb0[:, csl],
                                                         rhs=wc0l[:, r * D:(r + 1) * D],
                                                         start=(r == 0), stop=False)
                                        nc.tensor.matmul(outp[:, h, :], lhsT=asb1[0:c1, csl],
                                                         rhs=wc1l[:, r * D:(r + 1) * D],
                                                         start=False, stop=(r == R - 1 and h == nh - 1))
                                    if r == R - 1:
                                        for h in range(nh):
                                            nc.vector.tensor_tensor(
                                                out=stage[0:hb, bb * nh + h, 0:D],
                                                in0=outp[:, h, :],
                                                in1=brep[0:hb, :], op=mybir.AluOpType.add)
                    # stage -> DRAM shard (rows (b,p) -> b*hb+p, hb = blk//nh)
                    nh = 1 if blk <= 128 else 2
                    hb = blk // nh
                    oap = out_dram.ap().rearrange("(b p) d -> p b d", p=hb)
                    nc.sync.dma_start(out=oap, in_=stage[0:hb, :, :])

                layer(entp, it1, wt1, dt1, cfg.kt1, cfg.t1, gb1, cfg.nblk1, BLK1,
                      wc0[0], wc1[0], bbar[0], stage1, h1loc, lp)
                nc.gpsimd.collective_compute(
                    "AllGather", mybir.AluOpType.bypass, replica_groups=groups,
                    ins=[h1loc[:]], outs=[h1all[:]])
                layer(h1all, it2, wt2, dt2, cfg.kt2, cfg.t2, gb2, cfg.nblk2, 128,
                      wc0[1], wc1[1], bbar[1], stage2, h2loc, l2p)
                nc.gpsimd.collective_compute(
                    "AllGather", mybir.AluOpType.bypass, replica_groups=groups,
                    ins=[h2loc[:]], outs=[h2all[:]])

        # ---------------- classifier ----------------
        with ExitStack() as ctx:
            cl = ctx.enter_context(tc.tile_pool(name="cl", bufs=1))
            cps = ctx.enter_context(tc.tile_pool(name="cps", bufs=2, space="PSUM"))
            otp = ctx.enter_context(tc.tile_pool(name="otp", bufs=3))
            ident = cl.tile([128, 128], fdt)
            make_identity(nc, ident[:])

            nqt = cfg.nqt

            # idx tiles
            def load_q(dram, tag):
                q = cl.tile([128, cfg.b // 16], i16, tag=tag, name=tag)
                nc.sync.dma_start(out=q[:], in_=dram[:])
                return q

            qobj_sb = load_q(qobj, "qobj")
            qsub_sb = load_q(qsub, "qsub")
            qrel_sb = load_q(qrel, "qrel")

            def build_xt(qidx_t, remb, tag):
                """gather features, return XT chunk tiles [128, B] x4."""
                emb = cl.tile([128, nqt, DP], fdt, tag=f"emb{tag}", name=f"emb{tag}")
                nc.gpsimd.dma_gather(
                    out_ap=emb[:], in_ap=h2all[:], idxs_ap=qidx_t[:],
                    num_idxs=cfg.b, num_idxs_reg=cfg.b, elem_size=DP)
                rel_t = cl.tile([128, nqt, DP], fdt, tag=f"rel{tag}", name=f"rel{tag}")
                nc.gpsimd.dma_gather(
                    out_ap=rel_t[:], in_ap=remb[:], idxs_ap=qrel_sb[:],
                    num_idxs=cfg.b, num_idxs_reg=cfg.b, elem_size=DP)
                chunks = []
                for ci, (srct, f0) in enumerate(
                        ((emb, 0), (emb, 128), (rel_t, 0), (rel_t, 128))):
                    xt = cl.tile([128, cfg.b], fdt, tag=f"xt{tag}{ci}", name=f"xt{tag}{ci}")
                    for qt in range(nqt):
                        tp = cps.tile([128, 128], fdt, tag="tp", name="tp")
                        nc.tensor.transpose(tp[:], srct[:, qt, f0:f0 + 128], ident[:])
                        nc.scalar.copy(xt[:, qt * 128:(qt + 1) * 128], tp[:])
                    chunks.append(xt)
                return chunks

            # weights: 4 chunks of 128 rows (bias + ones-col folded by host)
            def load_w2(dram, tag):
                tiles = []
                for ci in range(4):
                    wt_ = cl.tile([128, cfg.vshard], fdt, tag=f"w{tag}{ci}", name=f"w{tag}{ci}")
                    nc.sync.dma_start(out=wt_[:], in_=dram[ci * 128:(ci + 1) * 128, :])
                    tiles.append(wt_)
                return tiles

            ws = load_w2(wsca, "s")
            wo = load_w2(woca, "o")

            xt_s = build_xt(qobj_sb, orelp, "s")   # sub_predict: [obj_emb|ore]
            xt_o = build_xt(qsub_sb, srelp, "o")   # obj_predict: [sub_emb|sre]

            for si, (xts, wts, outd) in enumerate(((xt_s, ws, slog),
                                                   (xt_o, wo, olog))):
                for (vo, vw) in cfg.vts:
                    for qt in range(nqt):
                        ps = cps.tile([128, vw], f32, tag="out", name="outc")
                        qsl = slice(qt * 128, (qt + 1) * 128)
                        for mi, (xt, wt_) in enumerate(zip(xts, wts)):
                            nc.tensor.matmul(ps[:], lhsT=xt[:, qsl],
                                             rhs=wt_[:, vo:vo + vw],
                                             start=(mi == 0), stop=(mi == 3))
                        ot = otp.tile([128, vw], fdt, tag="ot", name="ot")
                        if (qt + si) % 2 == 0:
                            nc.scalar.copy(ot[:], ps[:])
                        else:
                            nc.vector.tensor_copy(ot[:], ps[:])
                        nc.sync.dma_start(out=outd[qsl, vo:vo + vw], in_=ot[:])

    nc.compile()
    return nc


# ----------------------------------------------------------------------------
# entry point
# ----------------------------------------------------------------------------

def kernel(entity_emb, W0, b0, W1, b1, sub_rel_emb, obj_rel_emb,
           Wsc, bsc, Woc, boc, sub, obj, rel, src, dst, etype):
    entity_emb = np.asarray(entity_emb)
    nume, din = entity_emb.shape
    numr = np.asarray(sub_rel_emb).shape[0]
    e = np.asarray(src).shape[0]
    b = np.asarray(sub).shape[0]
    cfg = _make_cfg(nume, numr, din, e, b)
    sub = np.asarray(sub); obj = np.asarray(obj); rel = np.asarray(rel)
    src = np.asarray(src); dst = np.asarray(dst); etype = np.asarray(etype)

    meta = _host_prep(cfg, sub, obj, rel, src, dst, etype)
    for k_ in ("kt1", "t1", "kt2", "t2", "nblk2", "qshard", "qpad"):
        setattr(cfg, k_, meta[k_])

    key = (nume, numr, din, e, b, cfg.kt1, cfg.t1, cfg.kt2, cfg.t2, cfg.nblk2)
    if key not in _CACHE:
        _CACHE[key] = _build_nc(cfg)
    nc = _CACHE[key]

    # ---- host-side tensor prep (replicated & sharded inputs, bf16) ----
    f32 = np.float32

    def pad(a, ones_col=False):
        a = np.asarray(a, f32)
        p = np.zeros((a.shape[0], DP), f32)
        p[:, :a.shape[1]] = a
        if ones_col:
            p[:, a.shape[1]] = 1.0
        return p.astype(bf16)

    entp = pad(entity_emb)
    wch = np.stack([
        np.transpose(np.asarray(W0, f32), (1, 0, 2)).reshape(din, cfg.r * din),
        np.transpose(np.asarray(W1, f32), (1, 0, 2)).reshape(din, cfg.r * din),
    ]).astype(bf16)
    srelp = pad(np.asarray(sub_rel_emb, f32), ones_col=True)
    orelp = pad(np.asarray(obj_rel_emb, f32), ones_col=True)

    # classifier weights: 4 chunks of 128 rows matching XT layout
    # [W0:128 | W128:200+0pad | W200:328 | W328:400+bias@72+0pad]
    def waug(W, bias):
        W = np.asarray(W, f32)
        out = np.zeros((512, W.shape[1]), f32)
        out[0:128] = W[0:128]
        out[128:200] = W[128:200]
        out[256:384] = W[200:328]
        out[384:456] = W[328:400]
        out[456] = np.asarray(bias, f32)
        return out.astype(bf16)

    wsca = waug(Wsc, bsc)
    woca = waug(Woc, boc)

    in_maps = []
    V = cfg.vshard
    for c in range(NC):
        sl = slice(c * V, (c + 1) * V)
        in_maps.append(dict(
            entp=entp, wch=wch,
            b0=np.asarray(b0, f32), b1=np.asarray(b1, f32),
            wsca=np.ascontiguousarray(wsca[:, sl]),
            woca=np.ascontiguousarray(woca[:, sl]),
            srelp=srelp, orelp=orelp,
            idx1=meta["idx1"][c], wv1=meta["wv1"][c], dl1=meta["dl1"][c],
            idx2=meta["idx2"][c], wv2=meta["wv2"][c], dl2=meta["dl2"][c],
            qobj=meta["qobj"], qsub=meta["qsub"], qrel=meta["qrel"],
        ))

    res = run_bass_kernel_spmd(nc, in_maps, list(range(NC)),
                               trace=PROFILE, **(RUN_KWARGS or {}))
    kernel.last_result = res
    sub_p = np.concatenate([np.asarray(res.results[c]["slog"], f32) for c in range(NC)], axis=1)
    obj_p = np.concatenate([np.asarray(res.results[c]["olog"], f32) for c in range(NC)], axis=1)
    return (sub_p, obj_p)


# revision 12
# speedup vs baseline: 1.1098x; 1.1098x over previous
"""Trainium2 Bass kernel for nn_PreTrainModel_4355096838991 (gnn_message_passing).

2-layer hetero GraphConv (R=20 directed etypes) + link-prediction classifier.

Strategy (8 NeuronCores, SPMD single program):
  * Host (index-only prep): degree norms folded into per-edge weight
    w_e = src_norm[r,src]*dst_norm[r,dst]/R; all feature tensors pre-cast
    to bf16; dst nodes assigned to (core, 250-wide block) with a
    swap-repair balancer so every (core,block,rel) group fits kt=3 tiles
    of 128 edge slots (25% fewer gather descriptors than naive 125-wide
    blocks); pad slots fetch row 0 with w=0.
  * Device layer: dma_gather h[src] rows (256-col bf16 table), gathers
    alternating over 2 SWDGE queues to overlap per-descriptor HBM
    latency -> weighted one-hot O_w = (iota==dstloc)*w via one fused
    tensor_scalar per 128-slot tile -> scatter matmuls into PSUM
    accT[d_chunk, dst 250] -> transform accT @ W_r (two 125-halves)
    accumulated over relations in PSUM -> +mean-bias -> staged ->
    AllGather.
  * Layer 2 computes only the ~2048 queried nodes (sub/obj union).
  * Classifier: vocab-sharded GEMMs, features transposed via PE; bias and
    the rel-emb ones-column folded into the 4-chunk weight stack; bf16
    logit writes, host casts back to f32.
"""

import sys

for _p in ("/opt/trn_rl_repo/concourse", "/opt/trn_rl_repo"):
    if _p not in sys.path:
        sys.path.insert(0, _p)

import numpy as np
import ml_dtypes

import concourse.bacc as bacc
import concourse.bass as bass
import concourse.tile as tile
from concourse import mybir
from concourse.bass_utils import run_bass_kernel_spmd
from concourse.masks import make_identity

NC = 8
DP = 256  # padded feature row (bytes multiple of 256 for dma_gather)
TS_ = 128  # edge sub-tile (scatter matmul contraction K); 64 or 128
PROFILE = False
RUN_KWARGS = {}

_CACHE = {}
bf16 = ml_dtypes.bfloat16


def _largest_div(n, cap):
    for d in range(cap, 0, -1):
        if n % d == 0:
            return d
    return 1


class Cfg:
    pass


def _make_cfg(nume, numr, din, e, b):
    c = Cfg()
    c.nume, c.numr, c.r, c.d, c.e, c.b = nume, numr, 2 * numr, din, e, b
    assert din == 200 and nume % NC == 0 and nume <= 32000
    c.fdt = mybir.dt.bfloat16
    c.nshard = nume // NC
    c.blk1 = _largest_div(c.nshard, 250)     # 250 (double dst-block)
    c.nblk1 = c.nshard // c.blk1             # 10
    c.c0 = min(128, din)
    c.c1 = din - c.c0
    c.vshard = nume // NC
    c.nqt = b // 128
    assert b % 128 == 0
    c.vts = []
    off = 0
    while off < c.vshard:
        w = min(512, c.vshard - off)
        c.vts.append((off, w))
        off += w
    return c


def _balance(iv, nblocks, bsize, limit, iters=400):
    """Assign rows of iv [N, R] to nblocks blocks of exactly bsize each,
    swap-repairing until every (block, r) sum <= limit (best effort)."""
    n = iv.shape[0]
    assert n == nblocks * bsize
    assign = np.repeat(np.arange(nblocks), bsize)
    S = np.zeros((nblocks, iv.shape[1]), np.int64)
    np.add.at(S, assign, iv)
    for _ in range(iters):
        if S.max() <= limit:
            break
        b, r = np.unravel_index(S.argmax(), S.shape)
        mem = np.where(assign == b)[0]
        n_h = mem[iv[mem, r].argmax()]
        b2 = int(S[:, r].argmin())
        if b2 == b:
            break
        mem2 = np.where(assign == b2)[0]
        n_l = mem2[iv[mem2, r].argmin()]
        assign[n_h], assign[n_l] = b2, b
        S[b] += iv[n_l] - iv[n_h]
        S[b2] += iv[n_h] - iv[n_l]
    return assign


def _streams(gkey, src, w, dloc, kt, ngroups_per_core):
    """gkey: global group id per edge; groups padded to kt sub-tiles of 64."""
    slots_g = kt * TS_
    t64 = ngroups_per_core * kt
    slots = t64 * TS_
    assert (NC * slots) % 128 == 0
    order = np.argsort(gkey, kind="stable")
    ks = gkey[order]
    ngroups = NC * ngroups_per_core
    grp_start = np.searchsorted(ks, np.arange(ngroups))
    cnt = np.diff(np.append(grp_start, len(ks)))
    assert cnt.max() <= slots_g, (cnt.max(), slots_g)
    rank = np.arange(len(ks)) - grp_start[ks]
    glocal = ks % ngroups_per_core
    core_of = ks // ngroups_per_core
    pos = core_of * slots + glocal * slots_g + rank
    SRC = np.zeros(NC * slots, np.int16)
    WV = np.zeros(NC * slots, np.float32)
    DL = np.zeros(NC * slots, np.float32)
    SRC[pos] = src[order].astype(np.int16)
    WV[pos] = w[order]
    DL[pos] = dloc[order].astype(np.float32)
    t = slots // 128  # 128-columns per core
    SRC = SRC.reshape(NC, t, 128)
    WV = WV.reshape(NC, t, 128)
    DL = DL.reshape(NC, t, 128)
    idx = np.zeros((NC, 128, t * 8), np.int16)
    for c in range(NC):
        blk = SRC[c].reshape(t * 8, 16).T
        for k in range(8):  # replicate for the 8 GPSIMD descriptor cores
            idx[c, 16 * k:16 * k + 16] = blk
    wv = np.transpose(WV, (0, 2, 1)).copy()  # [NC, 128, T]
    dl = np.transpose(DL, (0, 2, 1)).copy()
    return idx, wv, dl, t


def _host_prep(cfg, sub, obj, rel, src, dst, etype):
    nume, R = cfg.nume, cfg.r
    seg_s = etype.astype(np.int64) * nume + src
    seg_d = etype.astype(np.int64) * nume + dst
    outdeg = np.bincount(seg_s, minlength=R * nume).astype(np.float32)
    indeg = np.bincount(seg_d, minlength=R * nume).astype(np.float32)
    srcn = np.maximum(outdeg, 1.0) ** -0.5
    dstn = np.maximum(indeg, 1.0) ** -0.5
    w = (srcn[seg_s] * dstn[seg_d] / R).astype(np.float32)

    iv = np.bincount(seg_d, minlength=R * nume).reshape(R, nume).T  # [N, R]

    # --- layer 1: balanced dst-node -> (core, block) assignment ---
    nblocks = NC * cfg.nblk1
    assignb = _balance(iv.astype(np.int64), nblocks, cfg.blk1, 3 * 128 - 6)
    # position of each node within the permuted table
    orderp = np.argsort(assignb, kind="stable")
    invpos = np.empty(nume, np.int64)
    invpos[orderp] = np.arange(nume)

    core1 = assignb[dst] // cfg.nblk1
    b1 = assignb[dst] % cfg.nblk1
    dloc1 = invpos[dst] % cfg.blk1
    gpc1 = cfg.nblk1 * R
    gkey1 = (core1.astype(np.int64) * cfg.nblk1 + b1) * R + etype
    cnt1 = np.bincount(gkey1, minlength=NC * gpc1)
    kt1 = int(-(-cnt1.max() // TS_))
    idx1, wv1, dl1, t1 = _streams(gkey1, src, w, dloc1, kt1, gpc1)

    # --- layer 2: only queried dst nodes, balanced into 16 q-blocks ---
    qn = np.unique(np.concatenate([sub, obj]))
    nq = len(qn)
    nblk2 = max(1, -(-nq // (128 * NC)))
    qshard = nblk2 * 128
    qpad = NC * qshard
    ivq = np.zeros((qpad, R), np.int64)
    ivq[:nq] = iv[qn]
    assignq = _balance(ivq, NC * nblk2, 128, 188)
    orderq = np.argsort(assignq, kind="stable")
    qpos_of = np.empty(qpad, np.int64)
    qpos_of[orderq] = np.arange(qpad)
    qmap = np.full(nume, -1, np.int64)
    qmap[qn] = qpos_of[np.arange(nq)]
    m = qmap[dst] >= 0
    q = qmap[dst[m]]
    core2 = q // qshard
    loc2 = q % qshard
    b2 = loc2 // 128
    dloc2 = loc2 % 128
    gpc2 = nblk2 * R
    gkey2 = (core2 * nblk2 + b2) * R + etype[m]
    cnt2 = np.bincount(gkey2, minlength=NC * gpc2)
    kt2 = max(2, int(-(-cnt2.max() // TS_)))
    # layer-2 gathers read the permuted h1 table
    idx2, wv2, dl2, t2 = _streams(gkey2, invpos[src[m]], w[m], dloc2, kt2, gpc2)

    # --- classifier query index streams (int16, [128, B/16] rows 0..15) ---
    def qidx(a):
        out = np.zeros((128, cfg.b // 16), np.int16)
        blk = a.astype(np.int16).reshape(cfg.b // 16, 16).T
        for k in range(8):
            out[16 * k:16 * k + 16] = blk
        return out

    meta = dict(
        kt1=kt1, t1=t1, kt2=kt2, t2=t2, nblk2=nblk2, qshard=qshard, qpad=qpad,
        idx1=idx1, wv1=wv1, dl1=dl1, idx2=idx2, wv2=wv2, dl2=dl2,
        qobj=qidx(qmap[obj]), qsub=qidx(qmap[sub]), qrel=qidx(rel),
        orderp=orderp,
    )
    return meta


# ----------------------------------------------------------------------------
# device program
# ----------------------------------------------------------------------------

def _build_nc(cfg):
    fdt = cfg.fdt
    f32 = mybir.dt.float32
    i16 = mybir.dt.int16
    R, D, BLK1 = cfg.r, cfg.d, cfg.blk1
    c0, c1 = cfg.c0, cfg.c1
    gb1 = _largest_div(cfg.t1, 8)  # dma_gather HW limit: 1024 idxs
    gb2 = _largest_div(cfg.t2, 8)

    nc = bacc.Bacc("TRN2", target_bir_lowering=False, debug=False, num_devices=NC,
                   num_swdge_queues=2)

    # ---- I/O (all feature data pre-cast to bf16 on host) ----
    entp = nc.dram_tensor("entp", [cfg.nume, DP], fdt, kind="ExternalInput")
    wch = nc.dram_tensor("wch", [2, D, R * D], fdt, kind="ExternalInput")
    b0 = nc.dram_tensor("b0", [R, D], f32, kind="ExternalInput")
    b1 = nc.dram_tensor("b1", [R, D], f32, kind="ExternalInput")
    wsca = nc.dram_tensor("wsca", [512, cfg.vshard], fdt, kind="ExternalInput")
    woca = nc.dram_tensor("woca", [512, cfg.vshard], fdt, kind="ExternalInput")
    srelp = nc.dram_tensor("srelp", [cfg.numr, DP], fdt, kind="ExternalInput")
    orelp = nc.dram_tensor("orelp", [cfg.numr, DP], fdt, kind="ExternalInput")
    idx1 = nc.dram_tensor("idx1", [128, cfg.t1 * 8], i16, kind="ExternalInput")
    wv1 = nc.dram_tensor("wv1", [128, cfg.t1], f32, kind="ExternalInput")
    dl1 = nc.dram_tensor("dl1", [128, cfg.t1], f32, kind="ExternalInput")
    idx2 = nc.dram_tensor("idx2", [128, cfg.t2 * 8], i16, kind="ExternalInput")
    wv2 = nc.dram_tensor("wv2", [128, cfg.t2], f32, kind="ExternalInput")
    dl2 = nc.dram_tensor("dl2", [128, cfg.t2], f32, kind="ExternalInput")
    qobj = nc.dram_tensor("qobj", [128, cfg.b // 16], i16, kind="ExternalInput")
    qsub = nc.dram_tensor("qsub", [128, cfg.b // 16], i16, kind="ExternalInput")
    qrel = nc.dram_tensor("qrel", [128, cfg.b // 16], i16, kind="ExternalInput")
    slog = nc.dram_tensor("slog", [cfg.b, cfg.vshard], fdt, kind="ExternalOutput")
    olog = nc.dram_tensor("olog", [cfg.b, cfg.vshard], fdt, kind="ExternalOutput")

    h1loc = nc.dram_tensor("h1loc", [cfg.nshard, DP], fdt)
    h1all = nc.dram_tensor("h1all", [cfg.nume, DP], fdt, addr_space="Shared")
    h2loc = nc.dram_tensor("h2loc", [cfg.qshard, DP], fdt)
    h2all = nc.dram_tensor("h2all", [cfg.qpad, DP], fdt, addr_space="Shared")

    groups = [list(range(NC))]

    with tile.TileContext(nc) as tc:
        from contextlib import ExitStack

        with ExitStack() as ctx:
            const = ctx.enter_context(tc.tile_pool(name="const", bufs=1))

            # constants
            iota_i = const.tile([128, 256], mybir.dt.int32)
            nc.gpsimd.iota(iota_i[:], pattern=[[1, 256]], base=0, channel_multiplier=0)
            iota_f = const.tile([128, 256], fdt)
            nc.vector.tensor_copy(iota_f[:], iota_i[:])

            # W chunk tiles (pre-cast on host, plain HWDGE loads)
            wc0 = [const.tile([c0, R * D], fdt, tag=f"wc0_{l}", name=f"wc0_{l}") for l in range(2)]
            wc1 = [const.tile([c1, R * D], fdt, tag=f"wc1_{l}", name=f"wc1_{l}") for l in range(2)]
            for l in range(2):
                nc.sync.dma_start(out=wc0[l][:], in_=wch[l, 0:c0, :])
                nc.sync.dma_start(out=wc1[l][:], in_=wch[l, c0:D, :])

            # metadata
            def meta_tiles(idx_d, wv_d, dl_d, t, nm):
                it = const.tile([128, t * 8], i16, tag=f"it{nm}", name=f"it{nm}")
                nc.sync.dma_start(out=it[:], in_=idx_d[:])
                wt = const.tile([128, t], f32, tag=f"wt{nm}", name=f"wt{nm}")
                nc.sync.dma_start(out=wt[:], in_=wv_d[:])
                dt_ = const.tile([128, t], f32, tag=f"dt{nm}", name=f"dt{nm}")
                nc.sync.dma_start(out=dt_[:], in_=dl_d[:])
                return it, wt, dt_

            it1, wt1, dt1 = meta_tiles(idx1, wv1, dl1, cfg.t1, "1")
            it2, wt2, dt2 = meta_tiles(idx2, wv2, dl2, cfg.t2, "2")

            stage1 = const.tile([128, cfg.nblk1 * max(1, cfg.blk1 // 128 + (1 if cfg.blk1 % 128 else 0)), DP], fdt)
            nc.vector.memset(stage1[:], 0.0)
            stage2 = const.tile([128, cfg.nblk2, DP], fdt)
            nc.vector.memset(stage2[:], 0.0)

            with (
                tc.tile_pool(name="lp", bufs=4) as lp,
                tc.tile_pool(name="l2p", bufs=4) as l2p,
                tc.tile_pool(name="ow", bufs=6) as owp,
                tc.tile_pool(name="acc", bufs=4) as accp,
                tc.tile_pool(name="ps", bufs=2, space="PSUM") as psp,
            ):
                # mean bias, replicated over partitions: ones[R,128].T @ b
                bbar = []
                ones_r = const.tile([R, 128], f32)
                nc.vector.memset(ones_r[:], 1.0 / R)
                for l, bt in enumerate((b0, b1)):
                    bsb = const.tile([R, D], f32, tag=f"bsb{l}", name=f"bsb{l}")
                    nc.sync.dma_start(out=bsb[:], in_=bt[:])
                    bps = psp.tile([128, D], f32, tag="outp", name="outp")
                    nc.tensor.matmul(bps[:], lhsT=ones_r[:], rhs=bsb[:], start=True, stop=True)
                    brep = const.tile([128, D], f32, tag=f"brep{l}", name=f"brep{l}")
                    nc.scalar.copy(brep[:], bps[:])
                    bbar.append(brep)

                def layer(tbl, it, wt, dt_, kt, t, gb, nblk, blk, wc0l, wc1l, brep,
                          stage, out_dram, pool):
                    nbat = t // gb
                    acc0 = acc1 = outp = None
                    for g in range(nbat):
                        msg = pool.tile([128, gb, DP], fdt, tag="msg", name="msg")
                        nc.gpsimd.dma_gather(
                            out_ap=msg[:],
                            in_ap=tbl[:],
                            idxs_ap=it[:, g * gb * 8:(g + 1) * gb * 8],
                            num_idxs=gb * 128,
                            num_idxs_reg=gb * 128,
                            elem_size=DP,
                            queue_num=g % 2,
                        )
                        for j in range(gb):
                            col = g * gb + j
                            ow = owp.tile([128, blk], fdt, tag="ow", name="ow")
                            nc.vector.tensor_scalar(
                                out=ow[:], in0=iota_f[:, :blk],
                                scalar1=dt_[:, col:col + 1], scalar2=wt[:, col:col + 1],
                                op0=mybir.AluOpType.is_equal, op1=mybir.AluOpType.mult,
                            )
                            for half in range(128 // TS_):
                                st = col * (128 // TS_) + half
                                grp = st // kt
                                k = st % kt
                                bb = grp // R
                                r = grp % R
                                if k == 0:
                                    acc0 = psp.tile([128, blk], f32, tag="accT0", name="accT0")
                                    acc1 = psp.tile([128, blk], f32, tag="accT1", name="accT1")
                                h0 = half * TS_
                                nc.tensor.matmul(acc0[:], lhsT=msg[h0:h0 + TS_, j, 0:c0],
                                                 rhs=ow[h0:h0 + TS_, :],
                                                 start=(k == 0), stop=(k == kt - 1))
                                # cols c0:c0+128 include the zero-padded tail so the
                                # full 128 psum partitions are written (rows >= c1: 0)
                                nc.tensor.matmul(acc1[:], lhsT=msg[h0:h0 + TS_, j, c0:c0 + 128],
                                                 rhs=ow[h0:h0 + TS_, :],
                                                 start=(k == 0), stop=(k == kt - 1))
                                if k == kt - 1:
                                    asb0 = accp.tile([128, blk], fdt, tag="asb0", name="asb0")
                                    asb1 = accp.tile([128, blk], fdt, tag="asb1", name="asb1")
                                    if grp % 2 == 0:
                                        nc.scalar.copy(asb0[:], acc0[:])
                                        nc.scalar.copy(asb1[:], acc1[:])
                                    else:
                                        nc.vector.tensor_copy(asb0[:], acc0[:])
                                        nc.vector.tensor_copy(asb1[:], acc1[:])
                                    nh = 1 if blk <= 128 else 2
                                    hb = blk // nh
                                    if r == 0:
                                        outp = psp.tile([hb, nh, D], f32, tag="outp", name="outp")
                                    for h in range(nh):
                                        csl = slice(h * hb, (h + 1) * hb)
                                        nc.tensor.matmul(outp[:, h, :], lhsT=asb0[:, csl],
                                                         rhs=wc0l[:, r * D:(r + 1) * D],
                                                         start=(r == 0), stop=False)
                                        nc.tensor.matmul(outp[:, h, :], lhsT=asb1[0:c1, csl],
                                                         rhs=wc1l[:, r * D:(r + 1) * D],
                                                         start=False, stop=(r == R - 1 and h == nh - 1))
                                    if r == R - 1:
                                        for h in range(nh):
                                            nc.vector.tensor_tensor(
                                                out=stage[0:hb, bb * nh + h, 0:D],
                                                in0=outp[:, h, :],
                                                in1=brep[0:hb, :], op=mybir.AluOpType.add)
                    # stage -> DRAM shard (rows (b,p) -> b*hb+p, hb = blk//nh)
                    nh = 1 if blk <= 128 else 2
                    hb = blk // nh
                    oap = out_dram.ap().rearrange("(b p) d -> p b d", p=hb)
                    nc.sync.dma_start(out=oap, in_=stage[0:hb, :, :])

                layer(entp, it1, wt1, dt1, cfg.kt1, cfg.t1, gb1, cfg.nblk1, BLK1,
                      wc0[0], wc1[0], bbar[0], stage1, h1loc, lp)
                nc.gpsimd.collective_compute(
                    "AllGather", mybir.AluOpType.bypass, replica_groups=groups,
                    ins=[h1loc[:]], outs=[h1all[:]])
                layer(h1all, it2, wt2, dt2, cfg.kt2, cfg.t2, gb2, cfg.nblk2, 128,
                      wc0[1], wc1[1], bbar[1], stage2, h2loc, l2p)
                nc.gpsimd.collective_compute(
                    "AllGather", mybir.AluOpType.bypass, replica_groups=groups,
                    ins=[h2loc[:]], outs=[h2all[:]])

        # ---------------- classifier ----------------
        with ExitStack() as ctx:
            cl = ctx.enter_context(tc.tile_pool(name="cl", bufs=1))
            cps = ctx.enter_context(tc.tile_pool(name="cps", bufs=2, space="PSUM"))
            otp = ctx.enter_context(tc.tile_pool(name="otp", bufs=3))
            ident = cl.tile([128, 128], fdt)
            make_identity(nc, ident[:])

            nqt = cfg.nqt

            # idx tiles
            def load_q(dram, tag):
                q = cl.tile([128, cfg.b // 16], i16, tag=tag, name=tag)
                nc.sync.dma_start(out=q[:], in_=dram[:])
                return q

            qobj_sb = load_q(qobj, "qobj")
            qsub_sb = load_q(qsub, "qsub")
            qrel_sb = load_q(qrel, "qrel")

            def build_xt(qidx_t, remb, tag):
                """gather features, return XT chunk tiles [128, B] x4."""
                emb = cl.tile([128, nqt, DP], fdt, tag=f"emb{tag}", name=f"emb{tag}")
                nc.gpsimd.dma_gather(
                    out_ap=emb[:], in_ap=h2all[:], idxs_ap=qidx_t[:],
                    num_idxs=cfg.b, num_idxs_reg=cfg.b, elem_size=DP)
                rel_t = cl.tile([128, nqt, DP], fdt, tag=f"rel{tag}", name=f"rel{tag}")
                nc.gpsimd.dma_gather(
                    out_ap=rel_t[:], in_ap=remb[:], idxs_ap=qrel_sb[:],
                    num_idxs=cfg.b, num_idxs_reg=cfg.b, elem_size=DP)
                chunks = []
                for ci, (srct, f0) in enumerate(
                        ((emb, 0), (emb, 128), (rel_t, 0), (rel_t, 128))):
                    xt = cl.tile([128, cfg.b], fdt, tag=f"xt{tag}{ci}", name=f"xt{tag}{ci}")
                    for qt in range(nqt):
                        tp = cps.tile([128, 128], fdt, tag="tp", name="tp")
                        nc.tensor.transpose(tp[:], srct[:, qt, f0:f0 + 128], ident[:])
                        nc.scalar.copy(xt[:, qt * 128:(qt + 1) * 128], tp[:])
                    chunks.append(xt)
                return chunks

            # weights: 4 chunks of 128 rows (bias + ones-col folded by host)
            def load_w2(dram, tag):
                tiles = []
                for ci in range(4):
                    wt_ = cl.tile([128, cfg.vshard], fdt, tag=f"w{tag}{ci}", name=f"w{tag}{ci}")
                    nc.sync.dma_start(out=wt_[:], in_=dram[ci * 128:(ci + 1) * 128, :])
                    tiles.append(wt_)
                return tiles

            ws = load_w2(wsca, "s")
            wo = load_w2(woca, "o")

            xt_s = build_xt(qobj_sb, orelp, "s")   # sub_predict: [obj_emb|ore]
            xt_o = build_xt(qsub_sb, srelp, "o")   # obj_predict: [sub_emb|sre]

            for si, (xts, wts, outd) in enumerate(((xt_s, ws, slog),
                                                   (xt_o, wo, olog))):
                for (vo, vw) in cfg.vts:
                    for qt in range(nqt):
                        ps = cps.tile([128, vw], f32, tag="out", name="outc")
                        qsl = slice(qt * 128, (qt + 1) * 128)
                        for mi, (xt, wt_) in enumerate(zip(xts, wts)):
                            nc.tensor.matmul(ps[:], lhsT=xt[:, qsl],
                                             rhs=wt_[:, vo:vo + vw],
                                             start=(mi == 0), stop=(mi == 3))
                        ot = otp.tile([128, vw], fdt, tag="ot", name="ot")
                        if (qt + si) % 2 == 0:
                            nc.scalar.copy(ot[:], ps[:])
                        else:
                            nc.vector.tensor_copy(ot[:], ps[:])
                        nc.sync.dma_start(out=outd[qsl, vo:vo + vw], in_=ot[:])

    nc.compile()
    return nc


# ----------------------------------------------------------------------------
# entry point
# ----------------------------------------------------------------------------

def kernel(entity_emb, W0, b0, W1, b1, sub_rel_emb, obj_rel_emb,
           Wsc, bsc, Woc, boc, sub, obj, rel, src, dst, etype):
    entity_emb = np.asarray(entity_emb)
    nume, din = entity_emb.shape
    numr = np.asarray(sub_rel_emb).shape[0]
    e = np.asarray(src).shape[0]
    b = np.asarray(sub).shape[0]
    cfg = _make_cfg(nume, numr, din, e, b)
    sub = np.asarray(sub); obj = np.asarray(obj); rel = np.asarray(rel)
    src = np.asarray(src); dst = np.asarray(dst); etype = np.asarray(etype)

    meta = _host_prep(cfg, sub, obj, rel, src, dst, etype)
    for k_ in ("kt1", "t1", "kt2", "t2", "nblk2", "qshard", "qpad"):
        setattr(cfg, k_, meta[k_])

    key = (nume, numr, din, e, b, cfg.kt1, cfg.t1, cfg.kt2, cfg.t2, cfg.nblk2)
    if key not in _CACHE:
        _CACHE[key] = _build_nc(cfg)
    nc = _CACHE[key]

    # ---- host-side tensor prep (replicated & sharded inputs, bf16) ----
    f32 = np.float32

    def pad(a, ones_col=False):
        a = np.asarray(a, f32)
        p = np.zeros((a.shape[0], DP), f32)
        p[:, :a.shape[1]] = a
        if ones_col:
            p[:, a.shape[1]] = 1.0
        return p.astype(bf16)

    entp = pad(entity_emb)
    wch = np.stack([
        np.transpose(np.asarray(W0, f32), (1, 0, 2)).reshape(din, cfg.r * din),
        np.transpose(np.asarray(W1, f32), (1, 0, 2)).reshape(din, cfg.r * din),
    ]).astype(bf16)
    srelp = pad(np.asarray(sub_rel_emb, f32), ones_col=True)
    orelp = pad(np.asarray(obj_rel_emb, f32), ones_col=True)

    # classifier weights: 4 chunks of 128 rows matching XT layout
    # [W0:128 | W128:200+0pad | W200:328 | W328:400+bias@72+0pad]
    def waug(W, bias):
        W = np.asarray(W, f32)
        out = np.zeros((512, W.shape[1]), f32)
        out[0:128] = W[0:128]
        out[128:200] = W[128:200]
        out[256:384] = W[200:328]
        out[384:456] = W[328:400]
        out[456] = np.asarray(bias, f32)
        return out.astype(bf16)

    wsca = waug(Wsc, bsc)
    woca = waug(Woc, boc)

    in_maps = []
    V = cfg.vshard
    for c in range(NC):
        sl = slice(c * V, (c + 1) * V)
        in_maps.append(dict(
            entp=entp, wch=wch,
            b0=np.asarray(b0, f32), b1=np.asarray(b1, f32),
            wsca=np.ascontiguousarray(wsca[:, sl]),
            woca=np.ascontiguousarray(woca[:, sl]),
            srelp=srelp, orelp=orelp,
            idx1=meta["idx1"][c], wv1=meta["wv1"][c], dl1=meta["dl1"][c],
            idx2=meta["idx2"][c], wv2=meta["wv2"][c], dl2=meta["dl2"][c],
            qobj=meta["qobj"], qsub=meta["qsub"], qrel=meta["qrel"],
        ))

    res = run_bass_kernel_spmd(nc, in_maps, list(range(NC)),
                               trace=PROFILE, **(RUN_KWARGS or {}))
    kernel.last_result = res
    sub_p = np.concatenate([np.asarray(res.results[c]["slog"], f32) for c in range(NC)], axis=1)
    obj_p = np.concatenate([np.asarray(res.results[c]["olog"], f32) for c in range(NC)], axis=1)
    return (sub_p, obj_p)
